# revision 77
# baseline (speedup 1.0000x reference)
"""Trainium2 Bass kernel for MultiHeadedAttention with learned per-key-position scaling.

Sharding over 8 NeuronCores: batch(2) x q-half(2) x head-half(2).
Each core: its batch's full keys/values, a 1024-row query slice, 6 heads.

Scores are computed transposed ([kpos, q]) so that:
  - the per-key-position divisor delta folds into the exp's per-partition scale,
  - the softmax denominator Z comes from a ones-column appended to V,
  - the P@V matmul runs "flipped": P chunks are the stationary operand and
    V-hat (V plus the ones column) streams, so each of the 16 key-chunk
    accumulation steps streams only 65 columns instead of replaying all
    1024 query columns (PE cost is proportional to streamed columns).

The flipped P@V produces x as [q, dv]; the softmax normalization 1/Z then
becomes a per-partition scalar multiply fused into the PSUM evacuation, and
the [q, dh] -> [dh, q] layout change for the output projection is done by the
XBAR DMA transpose engine (off the compute engines entirely).

The V-projection bias is folded out of the device kernel: x = P@(V0 + 1*bv^T)
normalizes to x0/Z + bv, so the host adds bv @ Wo into the output bias.

Precision: the q/k path (projections + scores) runs in float32r (PE's fast
rounded-fp32), value, attention probabilities and the output projection run
in bf16. delta is computed in fp32 from each core's own query slice and
exchanged between q-half partner cores with a tiny AllGather.
The host pre-transposes activations into [d_model, seq] layout so all device
DMAs are plain contiguous loads.

Host combines per-core partial outputs (sum over head-halves + bo').
"""

import sys

for _p in ("/opt/trn_rl_repo",):
    if _p not in sys.path:
        sys.path.insert(0, _p)

import numpy as np
import ml_dtypes

BF16 = ml_dtypes.bfloat16

B, S, D, H, DK = 2, 2048, 768, 12, 64
NCORES = 8
SQ = S // 2          # query rows per core
HH = H // 2          # heads per core
DH = HH * DK         # 384 head dims per core

_cache = {}


def _build(s=S, sq=SQ, hh=HH, d=D, dk=DK, n_qh=2, dbg=False):
    import concourse.bass as bass
    import concourse.mybir as mybir
    import concourse.tile as tile
    from concourse import bacc

    f32 = mybir.dt.float32
    f32r = mybir.dt.float32r
    bf = mybir.dt.bfloat16
    Exp = mybir.ActivationFunctionType.Exp
    mult = mybir.AluOpType.mult
    add = mybir.AluOpType.add
    amin = mybir.AluOpType.min
    amax = mybir.AluOpType.max

    dh = hh * dk
    KC = s // 128        # key-position chunks
    C6 = d // 128        # d_model chunks
    C3 = dh // 128       # output-dim chunks per core
    NQ = sq // 512       # 512-wide q column blocks (scores)
    QC = sq // 128       # q row chunks
    BW = 256             # streaming block width (projection inputs)
    NBK = s // BW        # key/value stream blocks
    NBQ = sq // BW       # query-slice stream blocks
    KCL = BW // 128      # kpos chunks per stream block

    # AllGather partners: cores sharing (batch, head-half), differing in
    # q-half; q-half 0 listed first so the gather lands in global key order.
    groups = [[b * 4 + hf, b * 4 + 2 + hf] for b in range(2) for hf in range(2)]
    if n_qh == 1:
        groups = None

    nc = bacc.Bacc("TRN2", target_bir_lowering=False, debug=False, num_devices=NCORES)

    t = {}
    t["qqT"] = nc.dram_tensor("qqT", [d, sq], f32r, kind="ExternalInput").ap()
    t["kT"] = nc.dram_tensor("kT", [d, s], f32r, kind="ExternalInput").ap()
    t["vT"] = nc.dram_tensor("vT", [d, s], bf, kind="ExternalInput").ap()
    t["maskT"] = nc.dram_tensor("maskT", [s, sq], bf, kind="ExternalInput").ap()
    t["wq"] = nc.dram_tensor("wq", [d, dh], f32r, kind="ExternalInput").ap()
    t["wk"] = nc.dram_tensor("wk", [d, dh], f32r, kind="ExternalInput").ap()
    t["wv"] = nc.dram_tensor("wv", [d, dh], bf, kind="ExternalInput").ap()
    t["wo"] = nc.dram_tensor("wo", [dh, d], bf, kind="ExternalInput").ap()
    t["wd"] = nc.dram_tensor("wd", [d, 1], f32, kind="ExternalInput").ap()
    t["bq"] = nc.dram_tensor("bq", [dh], f32, kind="ExternalInput").ap()
    t["bk"] = nc.dram_tensor("bk", [dh], f32, kind="ExternalInput").ap()
    t["bd"] = nc.dram_tensor("bd", [1], f32, kind="ExternalInput").ap()
    t["yp"] = nc.dram_tensor("yp", [sq, d], bf, kind="ExternalOutput").ap()
    if dbg:
        t["dxh"] = nc.dram_tensor("dxh", [128, sq // 128, hh, dk], bf, kind="ExternalOutput").ap()
        t["dxT"] = nc.dram_tensor("dxT", [128, (hh * dk) // 128, sq], bf, kind="ExternalOutput").ap()
        t["dps"] = nc.dram_tensor("dps", [128, s // 128, sq], bf, kind="ExternalOutput").ap()
        t["drz"] = nc.dram_tensor("drz", [128, hh, sq // 128], f32, kind="ExternalOutput").ap()

    # [d, *] tensors viewed as [128, C6, *] partition tiles
    def dview(ap):
        return ap.rearrange("(c p) s -> p c s", p=128)

    def bcast(ap, n):
        # broadcast a 1-D DRAM vector across n partitions
        return bass.AP(tensor=ap.tensor, offset=ap.offset, ap=[[0, n]] + list(ap.ap))

    with tile.TileContext(nc) as tc:
        with (
            tc.tile_pool(name="persist", bufs=1) as P,
            tc.tile_pool(name="pj", bufs=2, space="PSUM") as PJ,
            tc.tile_pool(name="xpp", bufs=3, space="PSUM") as XPP,
            tc.tile_pool(name="xv", bufs=1, space="PSUM") as XV,
            tc.tile_pool(name="work", bufs=1) as W,
            tc.tile_pool(name="work2", bufs=4) as W2,
            tc.tile_pool(name="load", bufs=4) as L,
            tc.tile_pool(name="loadfr", bufs=2) as LF,
            tc.tile_pool(name="dram", bufs=2, space="DRAM") as DR,
        ):
            maskT = P.tile([128, KC, sq], bf)
            vsb = P.tile([128, KC, hh, dk + 1], bf)
            # rolling store of masked attention probabilities: 16 live chunks
            # per head + 8 slots of slack so a head's P@V groups can drain
            # during the NEXT head's score/exp/mask stream without colliding
            PSS = KC + 8
            psS = P.tile([128, PSS, sq], bf)
            qTh = P.tile([128, C3, sq], f32r)    # head pairs packed on partitions
            kTh = P.tile([128, C3, s], f32r)
            xh = P.tile([128, QC, hh, dk], bf)   # attention out, [q, head, dv]
            xT = P.tile([128, C3, sq], bf)       # transposed for the out-proj
            wq_sb = P.tile([128, C6, dh], f32r)
            wk_sb = P.tile([128, C6, dh], f32r)
            wv_sb = P.tile([128, C6, dh], bf)
            wo_sb = P.tile([128, C3, d], bf)
            wd_sb = P.tile([128, C6, 1], f32)
            bqc = P.tile([128, C3], f32)
            bkc = P.tile([128, C3], f32)
            bdb = P.tile([128, 1], f32)
            rdcol = P.tile([128, KC], f32)

            # warm the ACT exp table while DMAs stream
            dummy = W.tile([1, 2], f32, tag="dummy")
            nc.vector.memset(dummy, 0.0)
            nc.scalar.activation(dummy, dummy, Exp, scale=1.0)

            nc.sync.dma_start(wd_sb, dview(t["wd"]))
            nc.gpsimd.dma_start(bdb, bcast(t["bd"], 128))
            nc.sync.dma_start(bqc, t["bq"].rearrange("(c p) -> p c", p=128))
            nc.vector.memset(vsb[:, :, :, dk : dk + 1], 1.0)

            def f32r_load(src_ap, blk):
                # direct f32r load (input tensors are declared f32r; the
                # PE rounds on read, verified on hardware)
                fr = L.tile([128, C6, BW], f32r, tag="ldf")
                nc.sync.dma_start(fr, src_ap[:, :, blk * BW : (blk + 1) * BW])
                return fr

            def mask_g(g):
                nc.sync.dma_start(
                    maskT[:, g * (KC // 4) : (g + 1) * (KC // 4), :],
                    t["maskT"].rearrange("(kc p) q -> p kc q", p=128)[
                        :, g * (KC // 4) : (g + 1) * (KC // 4), :
                    ],
                )

            # --- Q projection (+ local delta) over the query slice ---
            dps = XV.tile([128, sq // 128], f32, tag="vp")
            for blk in range(NBQ):
                qqb = f32r_load(dview(t["qqT"]), blk)
                qqf = qqb.bitcast(f32)
                if blk == 0:
                    # q/k weights ride behind the first query block
                    nc.sync.dma_start(wq_sb, dview(t["wq"]))
                    nc.sync.dma_start(wk_sb, dview(t["wk"]))
                    nc.sync.dma_start(bkc, t["bk"].rearrange("(c p) -> p c", p=128))
                if blk == NBQ - 1:
                    # k-block-0's load takes the DMA-pipe slot right after the
                    # last query block, ahead of the delta-exchange hops
                    k0_tile = f32r_load(dview(t["kT"]), 0)
                for kcl in range(KCL):
                    for c in range(C6):
                        nc.tensor.matmul(
                            dps[:, blk * KCL + kcl : blk * KCL + kcl + 1],
                            lhsT=qqf[:, c, kcl * 128 : (kcl + 1) * 128],
                            rhs=wd_sb[:, c, :],
                            start=(c == 0),
                            stop=(c == C6 - 1),
                        )
                for m in range(C3):
                    qp = PJ.tile([128, BW], f32, tag="pj")
                    for c in range(C6):
                        nc.tensor.matmul(
                            qp,
                            lhsT=wq_sb[:, c, m * 128 : (m + 1) * 128],
                            rhs=qqb[:, c, :],
                            start=(c == 0),
                            stop=(c == C6 - 1),
                        )
                    nc.vector.tensor_scalar_add(
                        out=qTh[:, m, blk * BW : (blk + 1) * BW],
                        in0=qp,
                        scalar1=bqc[:, m : m + 1],
                    )

            # local delta -> recip -> exchange with q-half partner
            dloc = W2.tile([128, sq // 128], f32, tag="dloc")
            nc.vector.tensor_scalar(
                out=dloc, in0=dps, scalar1=bdb, scalar2=0.0, op0=add, op1=amax
            )
            nc.vector.tensor_scalar(
                out=dloc, in0=dloc, scalar1=8.0, scalar2=1.0, op0=amin, op1=add
            )
            nc.vector.reciprocal(dloc, dloc)
            gin = DR.tile([sq], f32)
            nc.sync.dma_start(gin.rearrange("(kc p) -> p kc", p=128), dloc)
            if groups is not None:
                gout = DR.tile([s], f32)
                nc.gpsimd.collective_compute(
                    "AllGather",
                    mybir.AluOpType.bypass,
                    replica_groups=groups,
                    ins=[gin.opt()],
                    outs=[gout.opt()],
                )
            else:
                gout = DR.tile([s], f32)
                for qh in range(s // sq):
                    nc.sync.dma_start(gout[qh * sq : (qh + 1) * sq], gin)
            nc.sync.dma_start(rdcol, gout.rearrange("(kc p) -> p kc", p=128))

            nc.sync.dma_start(wv_sb, dview(t["wv"]))

            # stream emitters, interleaved with head-0 attention below
            def k_block(blk, pre=None):
                kfb = pre if pre is not None else f32r_load(dview(t["kT"]), blk)
                for m in range(C3):
                    kp = PJ.tile([128, BW], f32, tag="pj")
                    for c in range(C6):
                        nc.tensor.matmul(
                            kp,
                            lhsT=wk_sb[:, c, m * 128 : (m + 1) * 128],
                            rhs=kfb[:, c, :],
                            start=(c == 0),
                            stop=(c == C6 - 1),
                        )
                    nc.vector.tensor_scalar_add(
                        out=kTh[:, m, blk * BW : (blk + 1) * BW],
                        in0=kp,
                        scalar1=bkc[:, m : m + 1],
                    )

            def v_block(blk):
                vT = LF.tile([128, C6, BW], bf, tag="vb")
                nc.sync.dma_start(
                    vT, dview(t["vT"])[:, :, blk * BW : (blk + 1) * BW]
                )
                for kcl in range(KCL):
                    kc = blk * KCL + kcl
                    vp = XV.tile([128, dh], f32, tag="vp")
                    for c in range(C6):
                        nc.tensor.matmul(
                            vp,
                            lhsT=vT[:, c, kcl * 128 : (kcl + 1) * 128],
                            rhs=wv_sb[:, c, :],
                            start=(c == 0),
                            stop=(c == C6 - 1),
                        )
                    nc.vector.tensor_copy(
                        vsb[:, kc, :, 0:dk],
                        vp.rearrange("p (h e) -> p h e", h=hh),
                    )

            # --- attention; streams interleave with head 0 ---
            kc_per_blk = BW // 128
            LOOKAHEAD = 1
            k_block(0, pre=k0_tile)
            v_block(0)
            k_done = v_done = 1
            mask_g(0)
            m_done = 1

            def slot(hd, kc):
                return (KC * hd + kc) % PSS

            def pv_drain(hd, qc):
                # one flipped-P@V accumulation group (own PSUM bank), fused
                # normalize on evacuation, and the pair transpose once the
                # odd head of a pair is drained
                xq = XPP.tile([128, 512], f32, tag="xps")
                for kc in range(KC):
                    nc.tensor.matmul(
                        xq[:, 0 : dk + 1],
                        lhsT=psS[:, slot(hd, kc), qc * 128 : (qc + 1) * 128],
                        rhs=vsb[:, kc, hd, :],
                        start=(kc == 0),
                        stop=(kc == KC - 1),
                    )
                rz = W2.tile([128, 1], f32, tag="rz")
                nc.vector.reciprocal(rz, xq[:, dk : dk + 1])
                if dbg:
                    nc.sync.dma_start(t["drz"][:, hd, qc : qc + 1], rz)
                nc.vector.tensor_scalar_mul(
                    out=xh[:, qc, hd, :],
                    in0=xq[:, 0:dk],
                    scalar1=rz,
                )
                if hd % 2 == 1:
                    eng = nc.sync if qc % 2 == 0 else nc.scalar
                    eng.dma_start(
                        xT[:, hd // 2, qc * 128 : (qc + 1) * 128],
                        xh[:, qc, hd - 1 : hd + 1, :],
                        transpose=True,
                    )

            for h in range(hh):
                hoff = (h % 2) * 64
                for kc in range(KC):
                    sps = PJ.tile([128, sq], f32, tag="pj")
                    for nn in range(NQ):
                        nc.tensor.matmul(
                            sps[:, nn * 512 : (nn + 1) * 512],
                            lhsT=kTh[
                                hoff : hoff + 64, h // 2, kc * 128 : (kc + 1) * 128
                            ],
                            rhs=qTh[
                                hoff : hoff + 64, h // 2, nn * 512 : (nn + 1) * 512
                            ],
                            start=True,
                            stop=True,
                        )
                    psb = psS[:, slot(h, kc), :]
                    nc.scalar.activation(psb, sps, Exp, scale=rdcol[:, kc : kc + 1])
                    nc.vector.tensor_tensor(
                        out=psb, in0=psb, in1=maskT[:, kc, :], op=mult
                    )
                    if dbg and h == 1:
                        nc.sync.dma_start(t["dps"][:, kc, :], psb)
                    if h > 0 and kc < QC:
                        # drain the previous head's P@V while this head's
                        # scores/exp/mask stream keeps ACT and PE busy
                        pv_drain(h - 1, kc)
                    if h == 0:
                        j = kc // kc_per_blk + LOOKAHEAD
                        if kc % kc_per_blk == 0:
                            if j < NBK:
                                k_block(j); k_done += 1
                            gsz = KC // 4
                            while m_done < 4 and m_done <= (kc + kc_per_blk + gsz - 1) // gsz:
                                mask_g(m_done); m_done += 1
                        else:
                            if j < NBK:
                                v_block(j); v_done += 1
                        if kc == KC - 1:
                            while k_done < NBK:
                                k_block(k_done); k_done += 1
                            while v_done < NBK:
                                v_block(v_done); v_done += 1
                            nc.sync.dma_start(
                                wo_sb,
                                t["wo"].rearrange("(c p) m -> p c m", p=128),
                            )

            # --- epilogue: drain the last head per q-chunk, pipelined with
            # its pair transpose and the output projection (which starts its
            # accumulation with that freshest chunk: c order [C3-1, 0, 1])
            for qc in range(QC):
                pv_drain(hh - 1, qc)
                yps = PJ.tile([128, d], f32, tag="pj")
                corder = [C3 - 1] + list(range(C3 - 1))
                for ci, c in enumerate(corder):
                    for col in range(0, d, 512):
                        ncol = min(512, d - col)
                        nc.tensor.matmul(
                            yps[:, col : col + ncol],
                            lhsT=xT[:, c, qc * 128 : (qc + 1) * 128],
                            rhs=wo_sb[:, c, col : col + ncol],
                            start=(ci == 0),
                            stop=(ci == C3 - 1),
                        )
                ysb = W2.tile([128, d], bf, tag="ysb")
                if qc % 2 == 0:
                    nc.scalar.copy(ysb, yps)
                else:
                    nc.vector.tensor_copy(ysb, yps)
                nc.sync.dma_start(t["yp"][qc * 128 : (qc + 1) * 128, :], ysb)
            if dbg:
                nc.sync.dma_start(t["dxT"], xT)

    nc.compile()
    return nc


def _in_maps(query, key, value, mask, Wq, bq, Wk, bk, Wv, bv, Wo, Wd, bd, sq=SQ, dh=DH):
    query = np.asarray(query, np.float32)
    key = np.asarray(key, np.float32)
    value = np.asarray(value, np.float32)
    mask = np.asarray(mask)
    qT = [np.ascontiguousarray(query[b].T) for b in range(B)]
    kT = [np.ascontiguousarray(key[b].T) for b in range(B)]
    vT = [np.ascontiguousarray(value[b].T).astype(BF16) for b in range(B)]
    wqf = np.ascontiguousarray(Wq, np.float32)
    wkf = np.ascontiguousarray(Wk, np.float32)
    wvb = np.ascontiguousarray(Wv).astype(BF16)
    wob = np.ascontiguousarray(Wo).astype(BF16)
    wdf = np.ascontiguousarray(Wd, np.float32)
    bqf = np.ascontiguousarray(bq, np.float32)
    bkf = np.ascontiguousarray(bk, np.float32)
    bdf = np.ascontiguousarray(bd, np.float32)

    maps = []
    for c in range(NCORES):
        b, qh, hf = c // 4, (c // 2) % 2, c % 2
        qs = slice(qh * sq, (qh + 1) * sq)
        hs = slice(hf * dh, (hf + 1) * dh)
        maps.append(
            {
                "qqT": np.ascontiguousarray(qT[b][:, qs]),
                "kT": kT[b],
                "vT": vT[b],
                "maskT": np.ascontiguousarray(mask[b, qs].T).astype(BF16),
                "wq": np.ascontiguousarray(wqf[:, hs]),
                "wk": np.ascontiguousarray(wkf[:, hs]),
                "wv": np.ascontiguousarray(wvb[:, hs]),
                "wo": np.ascontiguousarray(wob[hs, :]),
                "wd": wdf,
                "bq": np.ascontiguousarray(bqf[hs]),
                "bk": np.ascontiguousarray(bkf[hs]),
                "bd": bdf,
            }
        )
    return maps


def kernel(query, key, value, mask, Wq, bq, Wk, bk, Wv, bv, Wo, bo, Wd, bd):
    from concourse.bass_utils import run_bass_kernel_spmd

    if "nc" not in _cache:
        _cache["nc"] = _build()
    nc = _cache["nc"]

    maps = _in_maps(query, key, value, mask, Wq, bq, Wk, bk, Wv, bv, Wo, Wd, bd)
    res = run_bass_kernel_spmd(nc, maps, core_ids=list(range(NCORES)))

    # v-projection bias folded into the output bias: x = P@(V0 + 1*bv^T)
    # normalizes to x0/Z + bv, and (x0 + bv) @ Wo + bo = x0 @ Wo + bo'
    bof = np.asarray(bv, np.float32) @ np.asarray(Wo, np.float32) + np.asarray(
        bo, np.float32
    )
    y = np.empty((B, S, D), np.float32)
    for b in range(B):
        for qh in range(2):
            c0 = b * 4 + qh * 2
            y[b, qh * SQ : (qh + 1) * SQ] = (
                res.results[c0]["yp"].astype(np.float32)
                + res.results[c0 + 1]["yp"].astype(np.float32)
                + bof[None, :]
            )
    return y


# revision 85
# speedup vs baseline: 1.0058x; 1.0058x over previous
"""Trainium2 Bass kernel for MultiHeadedAttention with learned per-key-position scaling.

Sharding over 8 NeuronCores: batch(2) x q-half(2) x head-half(2).
Each core: its batch's full keys/values, a 1024-row query slice, 6 heads.

Scores are computed transposed ([kpos, q]) so that:
  - the per-key-position divisor delta folds into the exp's per-partition scale,
  - the softmax denominator Z comes from a ones-column appended to V,
  - the P@V matmul runs "flipped": P chunks are the stationary operand and
    V-hat (V plus the ones column) streams, so each of the 16 key-chunk
    accumulation steps streams only 65 columns instead of replaying all
    1024 query columns (PE cost is proportional to streamed columns).

The flipped P@V produces x as [q, dv]; the softmax normalization 1/Z then
becomes a per-partition scalar multiply fused into the PSUM evacuation, and
the [q, dh] -> [dh, q] layout change for the output projection is done by the
XBAR DMA transpose engine (off the compute engines entirely).

The V-projection bias is folded out of the device kernel: x = P@(V0 + 1*bv^T)
normalizes to x0/Z + bv, so the host adds bv @ Wo into the output bias.

Precision: the q/k path (projections + scores) runs in float32r (PE's fast
rounded-fp32), value, attention probabilities and the output projection run
in bf16. delta is computed in fp32 from each core's own query slice and
exchanged between q-half partner cores with a tiny AllGather.
The host pre-transposes activations into [d_model, seq] layout so all device
DMAs are plain contiguous loads.

Host combines per-core partial outputs (sum over head-halves + bo').
"""

import sys

for _p in ("/opt/trn_rl_repo",):
    if _p not in sys.path:
        sys.path.insert(0, _p)

import numpy as np
import ml_dtypes

BF16 = ml_dtypes.bfloat16

B, S, D, H, DK = 2, 2048, 768, 12, 64
NCORES = 8
SQ = S // 2          # query rows per core
HH = H // 2          # heads per core
DH = HH * DK         # 384 head dims per core

_cache = {}


def _build(s=S, sq=SQ, hh=HH, d=D, dk=DK, n_qh=2, dbg=False):
    import concourse.bass as bass
    import concourse.mybir as mybir
    import concourse.tile as tile
    from concourse import bacc

    f32 = mybir.dt.float32
    f32r = mybir.dt.float32r
    bf = mybir.dt.bfloat16
    Exp = mybir.ActivationFunctionType.Exp
    mult = mybir.AluOpType.mult
    add = mybir.AluOpType.add
    amin = mybir.AluOpType.min
    amax = mybir.AluOpType.max

    dh = hh * dk
    KC = s // 128        # key-position chunks
    C6 = d // 128        # d_model chunks
    C3 = dh // 128       # output-dim chunks per core
    NQ = sq // 512       # 512-wide q column blocks (scores)
    QC = sq // 128       # q row chunks
    BW = 256             # streaming block width (projection inputs)
    NBK = s // BW        # key/value stream blocks
    NBQ = sq // BW       # query-slice stream blocks
    KCL = BW // 128      # kpos chunks per stream block

    # AllGather partners: cores sharing (batch, head-half), differing in
    # q-half; q-half 0 listed first so the gather lands in global key order.
    groups = [[b * 4 + hf, b * 4 + 2 + hf] for b in range(2) for hf in range(2)]
    if n_qh == 1:
        groups = None

    nc = bacc.Bacc("TRN2", target_bir_lowering=False, debug=False, num_devices=NCORES)

    t = {}
    t["qqT"] = nc.dram_tensor("qqT", [d, sq], f32r, kind="ExternalInput").ap()
    t["kT"] = nc.dram_tensor("kT", [d, s], f32r, kind="ExternalInput").ap()
    t["vT"] = nc.dram_tensor("vT", [d, s], bf, kind="ExternalInput").ap()
    t["maskT"] = nc.dram_tensor("maskT", [s, sq], bf, kind="ExternalInput").ap()
    t["wq"] = nc.dram_tensor("wq", [d, dh], f32r, kind="ExternalInput").ap()
    t["wk"] = nc.dram_tensor("wk", [d, dh], f32r, kind="ExternalInput").ap()
    t["wv"] = nc.dram_tensor("wv", [d, dh], bf, kind="ExternalInput").ap()
    t["wo"] = nc.dram_tensor("wo", [dh, d], bf, kind="ExternalInput").ap()
    t["wd"] = nc.dram_tensor("wd", [d, 1], f32, kind="ExternalInput").ap()
    t["bq"] = nc.dram_tensor("bq", [dh], f32, kind="ExternalInput").ap()
    t["bk"] = nc.dram_tensor("bk", [dh], f32, kind="ExternalInput").ap()
    t["bd"] = nc.dram_tensor("bd", [1], f32, kind="ExternalInput").ap()
    t["yp"] = nc.dram_tensor("yp", [sq, d], bf, kind="ExternalOutput").ap()
    if dbg:
        t["dxh"] = nc.dram_tensor("dxh", [128, sq // 128, hh, dk], bf, kind="ExternalOutput").ap()
        t["dxT"] = nc.dram_tensor("dxT", [128, (hh * dk) // 128, sq], bf, kind="ExternalOutput").ap()
        t["dps"] = nc.dram_tensor("dps", [128, s // 128, sq], bf, kind="ExternalOutput").ap()
        t["drz"] = nc.dram_tensor("drz", [128, hh, sq // 128], f32, kind="ExternalOutput").ap()

    # [d, *] tensors viewed as [128, C6, *] partition tiles
    def dview(ap):
        return ap.rearrange("(c p) s -> p c s", p=128)

    def bcast(ap, n):
        # broadcast a 1-D DRAM vector across n partitions
        return bass.AP(tensor=ap.tensor, offset=ap.offset, ap=[[0, n]] + list(ap.ap))

    with tile.TileContext(nc) as tc:
        with (
            tc.tile_pool(name="persist", bufs=1) as P,
            tc.tile_pool(name="pj", bufs=2, space="PSUM") as PJ,
            tc.tile_pool(name="xpp", bufs=3, space="PSUM") as XPP,
            tc.tile_pool(name="xv", bufs=1, space="PSUM") as XV,
            tc.tile_pool(name="work", bufs=1) as W,
            tc.tile_pool(name="work2", bufs=4) as W2,
            tc.tile_pool(name="load", bufs=4) as L,
            tc.tile_pool(name="loadfr", bufs=2) as LF,
            tc.tile_pool(name="dram", bufs=2, space="DRAM") as DR,
        ):
            maskT = P.tile([128, KC, sq], bf)
            vsb = P.tile([128, KC, hh, dk + 1], bf)
            # rolling store of masked attention probabilities: 16 live chunks
            # per head + 8 slots of slack so a head's P@V groups can drain
            # during the NEXT head's score/exp/mask stream without colliding
            PSS = KC + 8
            psS = P.tile([128, PSS, sq], bf)
            qTh = P.tile([128, C3, sq], f32r)    # head pairs packed on partitions
            kTh = P.tile([128, C3, s], f32r)
            xh = P.tile([128, QC, hh, dk], bf)   # attention out, [q, head, dv]
            xT = P.tile([128, C3, sq], bf)       # transposed for the out-proj
            wq_sb = P.tile([128, C6, dh], f32r)
            wk_sb = P.tile([128, C6, dh], f32r)
            wv_sb = P.tile([128, C6, dh], bf)
            wo_sb = P.tile([128, C3, d], bf)
            wd_sb = P.tile([128, C6, 1], f32)
            bqc = P.tile([128, C3], f32)
            bkc = P.tile([128, C3], f32)
            bdb = P.tile([128, 1], f32)
            rdcol = P.tile([128, KC], f32)

            # warm the ACT exp table while DMAs stream
            dummy = W.tile([1, 2], f32, tag="dummy")
            nc.vector.memset(dummy, 0.0)
            nc.scalar.activation(dummy, dummy, Exp, scale=1.0)

            # small-vector loads dispatch from the (idle) ACT sequencer so
            # the SP sequencer's 650ns-per-dispatch serialization starts with
            # the critical qq/wq stream instead
            nc.scalar.dma_start(wd_sb, dview(t["wd"]))
            nc.gpsimd.dma_start(bdb, bcast(t["bd"], 128))
            nc.scalar.dma_start(bqc, t["bq"].rearrange("(c p) -> p c", p=128))
            nc.vector.memset(vsb[:, :, :, dk : dk + 1], 1.0)

            def f32r_load(src_ap, blk):
                # direct f32r load (input tensors are declared f32r; the
                # PE rounds on read, verified on hardware)
                fr = L.tile([128, C6, BW], f32r, tag="ldf")
                nc.sync.dma_start(fr, src_ap[:, :, blk * BW : (blk + 1) * BW])
                return fr

            def mask_g(g):
                nc.sync.dma_start(
                    maskT[:, g * (KC // 4) : (g + 1) * (KC // 4), :],
                    t["maskT"].rearrange("(kc p) q -> p kc q", p=128)[
                        :, g * (KC // 4) : (g + 1) * (KC // 4), :
                    ],
                )

            # --- Q projection (+ local delta) over the query slice ---
            dps = XV.tile([128, sq // 128], f32, tag="vp")
            for blk in range(NBQ):
                qqb = f32r_load(dview(t["qqT"]), blk)
                qqf = qqb.bitcast(f32)
                if blk == 0:
                    # q/k weights ride behind the first query block
                    nc.sync.dma_start(wq_sb, dview(t["wq"]))
                    nc.sync.dma_start(wk_sb, dview(t["wk"]))
                    nc.scalar.dma_start(bkc, t["bk"].rearrange("(c p) -> p c", p=128))
                if blk == NBQ - 1:
                    # k-block-0's load takes the DMA-pipe slot right after the
                    # last query block, ahead of the delta-exchange hops
                    k0_tile = f32r_load(dview(t["kT"]), 0)
                for kcl in range(KCL):
                    for c in range(C6):
                        nc.tensor.matmul(
                            dps[:, blk * KCL + kcl : blk * KCL + kcl + 1],
                            lhsT=qqf[:, c, kcl * 128 : (kcl + 1) * 128],
                            rhs=wd_sb[:, c, :],
                            start=(c == 0),
                            stop=(c == C6 - 1),
                        )
                for m in range(C3):
                    qp = PJ.tile([128, BW], f32, tag="pj")
                    for c in range(C6):
                        nc.tensor.matmul(
                            qp,
                            lhsT=wq_sb[:, c, m * 128 : (m + 1) * 128],
                            rhs=qqb[:, c, :],
                            start=(c == 0),
                            stop=(c == C6 - 1),
                        )
                    nc.vector.tensor_scalar_add(
                        out=qTh[:, m, blk * BW : (blk + 1) * BW],
                        in0=qp,
                        scalar1=bqc[:, m : m + 1],
                    )

            # local delta -> recip -> exchange with q-half partner
            dloc = W2.tile([128, sq // 128], f32, tag="dloc")
            nc.vector.tensor_scalar(
                out=dloc, in0=dps, scalar1=bdb, scalar2=0.0, op0=add, op1=amax
            )
            nc.vector.tensor_scalar(
                out=dloc, in0=dloc, scalar1=8.0, scalar2=1.0, op0=amin, op1=add
            )
            nc.vector.reciprocal(dloc, dloc)
            gin = DR.tile([sq], f32)
            nc.sync.dma_start(gin.rearrange("(kc p) -> p kc", p=128), dloc)
            if groups is not None:
                gout = DR.tile([s], f32)
                nc.gpsimd.collective_compute(
                    "AllGather",
                    mybir.AluOpType.bypass,
                    replica_groups=groups,
                    ins=[gin.opt()],
                    outs=[gout.opt()],
                )
            else:
                gout = DR.tile([s], f32)
                for qh in range(s // sq):
                    nc.sync.dma_start(gout[qh * sq : (qh + 1) * sq], gin)
            nc.sync.dma_start(rdcol, gout.rearrange("(kc p) -> p kc", p=128))

            nc.sync.dma_start(wv_sb, dview(t["wv"]))

            # stream emitters, interleaved with head-0 attention below
            def k_block(blk, pre=None):
                kfb = pre if pre is not None else f32r_load(dview(t["kT"]), blk)
                for m in range(C3):
                    kp = PJ.tile([128, BW], f32, tag="pj")
                    for c in range(C6):
                        nc.tensor.matmul(
                            kp,
                            lhsT=wk_sb[:, c, m * 128 : (m + 1) * 128],
                            rhs=kfb[:, c, :],
                            start=(c == 0),
                            stop=(c == C6 - 1),
                        )
                    nc.vector.tensor_scalar_add(
                        out=kTh[:, m, blk * BW : (blk + 1) * BW],
                        in0=kp,
                        scalar1=bkc[:, m : m + 1],
                    )

            def v_block(blk):
                vT = LF.tile([128, C6, BW], bf, tag="vb")
                nc.sync.dma_start(
                    vT, dview(t["vT"])[:, :, blk * BW : (blk + 1) * BW]
                )
                for kcl in range(KCL):
                    kc = blk * KCL + kcl
                    vp = XV.tile([128, dh], f32, tag="vp")
                    for c in range(C6):
                        nc.tensor.matmul(
                            vp,
                            lhsT=vT[:, c, kcl * 128 : (kcl + 1) * 128],
                            rhs=wv_sb[:, c, :],
                            start=(c == 0),
                            stop=(c == C6 - 1),
                        )
                    nc.vector.tensor_copy(
                        vsb[:, kc, :, 0:dk],
                        vp.rearrange("p (h e) -> p h e", h=hh),
                    )

            # --- attention; streams interleave with head 0 ---
            kc_per_blk = BW // 128
            LOOKAHEAD = 1
            k_block(0, pre=k0_tile)
            v_block(0)
            k_done = v_done = 1
            mask_g(0)
            m_done = 1

            def slot(hd, kc):
                return (KC * hd + kc) % PSS

            def pv_drain(hd, qc):
                # one flipped-P@V accumulation group (own PSUM bank), fused
                # normalize on evacuation, and the pair transpose once the
                # odd head of a pair is drained
                xq = XPP.tile([128, 512], f32, tag="xps")
                for kc in range(KC):
                    nc.tensor.matmul(
                        xq[:, 0 : dk + 1],
                        lhsT=psS[:, slot(hd, kc), qc * 128 : (qc + 1) * 128],
                        rhs=vsb[:, kc, hd, :],
                        start=(kc == 0),
                        stop=(kc == KC - 1),
                    )
                rz = W2.tile([128, 1], f32, tag="rz")
                nc.vector.reciprocal(rz, xq[:, dk : dk + 1])
                if dbg:
                    nc.sync.dma_start(t["drz"][:, hd, qc : qc + 1], rz)
                nc.vector.tensor_scalar_mul(
                    out=xh[:, qc, hd, :],
                    in0=xq[:, 0:dk],
                    scalar1=rz,
                )
                if hd % 2 == 1:
                    # ACT-queue dispatch only in the epilogue (ACT idle); the
                    # mid-window pairs must not steal ACT.SEQ slots from exps
                    eng = nc.scalar if (hd == hh - 1 and qc % 2 == 1) else nc.sync
                    eng.dma_start(
                        xT[:, hd // 2, qc * 128 : (qc + 1) * 128],
                        xh[:, qc, hd - 1 : hd + 1, :],
                        transpose=True,
                    )

            for h in range(hh):
                hoff = (h % 2) * 64
                for kc in range(KC):
                    sps = PJ.tile([128, sq], f32, tag="pj")
                    for nn in range(NQ):
                        nc.tensor.matmul(
                            sps[:, nn * 512 : (nn + 1) * 512],
                            lhsT=kTh[
                                hoff : hoff + 64, h // 2, kc * 128 : (kc + 1) * 128
                            ],
                            rhs=qTh[
                                hoff : hoff + 64, h // 2, nn * 512 : (nn + 1) * 512
                            ],
                            start=True,
                            stop=True,
                        )
                    psb = psS[:, slot(h, kc), :]
                    nc.scalar.activation(psb, sps, Exp, scale=rdcol[:, kc : kc + 1])
                    nc.vector.tensor_tensor(
                        out=psb, in0=psb, in1=maskT[:, kc, :], op=mult
                    )
                    if dbg and h == 1:
                        nc.sync.dma_start(t["dps"][:, kc, :], psb)
                    if h > 0 and kc < QC:
                        # drain the previous head's P@V while this head's
                        # scores/exp/mask stream keeps ACT and PE busy
                        pv_drain(h - 1, kc)
                    if h == 0:
                        j = kc // kc_per_blk + LOOKAHEAD
                        if kc % kc_per_blk == 0:
                            if j < NBK:
                                k_block(j); k_done += 1
                            gsz = KC // 4
                            while m_done < 4 and m_done <= (kc + kc_per_blk + gsz - 1) // gsz:
                                mask_g(m_done); m_done += 1
                        else:
                            if j < NBK:
                                v_block(j); v_done += 1
                        if kc == KC - 1:
                            while k_done < NBK:
                                k_block(k_done); k_done += 1
                            while v_done < NBK:
                                v_block(v_done); v_done += 1
                            nc.sync.dma_start(
                                wo_sb,
                                t["wo"].rearrange("(c p) m -> p c m", p=128),
                            )

            # --- epilogue: drain the last head per q-chunk, pipelined with
            # its pair transpose and the output projection (which starts its
            # accumulation with that freshest chunk: c order [C3-1, 0, 1])
            for qc in range(QC):
                pv_drain(hh - 1, qc)
                yps = PJ.tile([128, d], f32, tag="pj")
                corder = [C3 - 1] + list(range(C3 - 1))
                for ci, c in enumerate(corder):
                    for col in range(0, d, 512):
                        ncol = min(512, d - col)
                        nc.tensor.matmul(
                            yps[:, col : col + ncol],
                            lhsT=xT[:, c, qc * 128 : (qc + 1) * 128],
                            rhs=wo_sb[:, c, col : col + ncol],
                            start=(ci == 0),
                            stop=(ci == C3 - 1),
                        )
                ysb = W2.tile([128, d], bf, tag="ysb")
                if qc % 2 == 0:
                    nc.scalar.copy(ysb, yps)
                else:
                    nc.vector.tensor_copy(ysb, yps)
                nc.sync.dma_start(t["yp"][qc * 128 : (qc + 1) * 128, :], ysb)
            if dbg:
                nc.sync.dma_start(t["dxT"], xT)

    nc.compile()
    return nc


def _in_maps(query, key, value, mask, Wq, bq, Wk, bk, Wv, bv, Wo, Wd, bd, sq=SQ, dh=DH):
    query = np.asarray(query, np.float32)
    key = np.asarray(key, np.float32)
    value = np.asarray(value, np.float32)
    mask = np.asarray(mask)
    qT = [np.ascontiguousarray(query[b].T) for b in range(B)]
    kT = [np.ascontiguousarray(key[b].T) for b in range(B)]
    vT = [np.ascontiguousarray(value[b].T).astype(BF16) for b in range(B)]
    wqf = np.ascontiguousarray(Wq, np.float32)
    wkf = np.ascontiguousarray(Wk, np.float32)
    wvb = np.ascontiguousarray(Wv).astype(BF16)
    wob = np.ascontiguousarray(Wo).astype(BF16)
    wdf = np.ascontiguousarray(Wd, np.float32)
    bqf = np.ascontiguousarray(bq, np.float32)
    bkf = np.ascontiguousarray(bk, np.float32)
    bdf = np.ascontiguousarray(bd, np.float32)

    maps = []
    for c in range(NCORES):
        b, qh, hf = c // 4, (c // 2) % 2, c % 2
        qs = slice(qh * sq, (qh + 1) * sq)
        hs = slice(hf * dh, (hf + 1) * dh)
        maps.append(
            {
                "qqT": np.ascontiguousarray(qT[b][:, qs]),
                "kT": kT[b],
                "vT": vT[b],
                "maskT": np.ascontiguousarray(mask[b, qs].T).astype(BF16),
                "wq": np.ascontiguousarray(wqf[:, hs]),
                "wk": np.ascontiguousarray(wkf[:, hs]),
                "wv": np.ascontiguousarray(wvb[:, hs]),
                "wo": np.ascontiguousarray(wob[hs, :]),
                "wd": wdf,
                "bq": np.ascontiguousarray(bqf[hs]),
                "bk": np.ascontiguousarray(bkf[hs]),
                "bd": bdf,
            }
        )
    return maps


def kernel(query, key, value, mask, Wq, bq, Wk, bk, Wv, bv, Wo, bo, Wd, bd):
    from concourse.bass_utils import run_bass_kernel_spmd

    if "nc" not in _cache:
        _cache["nc"] = _build()
    nc = _cache["nc"]

    maps = _in_maps(query, key, value, mask, Wq, bq, Wk, bk, Wv, bv, Wo, Wd, bd)
    res = run_bass_kernel_spmd(nc, maps, core_ids=list(range(NCORES)))

    # v-projection bias folded into the output bias: x = P@(V0 + 1*bv^T)
    # normalizes to x0/Z + bv, and (x0 + bv) @ Wo + bo = x0 @ Wo + bo'
    bof = np.asarray(bv, np.float32) @ np.asarray(Wo, np.float32) + np.asarray(
        bo, np.float32
    )
    y = np.empty((B, S, D), np.float32)
    for b in range(B):
        for qh in range(2):
            c0 = b * 4 + qh * 2
            y[b, qh * SQ : (qh + 1) * SQ] = (
                res.results[c0]["yp"].astype(np.float32)
                + res.results[c0 + 1]["yp"].astype(np.float32)
                + bof[None, :]
            )
    return y
